# revision 23
# baseline (speedup 1.0000x reference)
"""Deformable-alignment kernel for Trainium2 (8 NeuronCores, batch-parallel).

Per core (one batch item):
  A. Pad x/ref to [128, 98*98] bf16 for the conv; build an unpadded
     pair-expanded gather source rsrc[c, i] = (ref[i], ref[i+1]) as bf16 pairs
     packed in f32 elements (ap_gather cost counts elements, so f32-packed
     pairs halve the source-scan cost).
  B. Offset/modulator conv (27 ch) as shift-im2col bf16 matmuls; transposed
     pixel-major directly into a PSUM-resident PPIX [128 px, 72, 27].
  C. Batched pixel pipeline: 18-wide (y&x stacked) ops over all 9 taps at
     once; coefs carry the validity masks and the 2x-sigmoid modulator;
     coef pairs and flat gather indices round-trip DRAM in q-order.
  D. Per (tap, corner-row): ONE full-image ap_gather (9216 idxs, d=1 f32
     pairs).  Coef pairs replicated to 128 partitions per quarter-image via
     either broadcast-DMA or in-place gpsimd partition_broadcast (split
     tuned so Pool and the DMA ring finish together).  DVE multiplies in
     place (bf16 4x mode), sums the two corner rows, and the PE contracts
     per-tap with accumulation over taps carried in a bf16 partial
     (Act evict + DVE add), since PSUM cannot hold the full image.
"""
import sys

sys.path.insert(0, "/opt/trn_rl_repo")

import numpy as np

import concourse.bass as bass
import concourse.bacc as bacc
import concourse.mybir as mybir
from concourse.tile import TileContext
from concourse.bass_utils import run_bass_kernel_spmd

B, C, H, W = 8, 128, 96, 96
HW = H * W                      # 9216
PH, PW = H + 2, W + 2
PHW = PH * PW                   # 9604
KH = KW = 3
K = KH * KW
CO = 27
NCH = 24
CHW = HW // NCH                 # 384
NF = HW // 128                  # 72
NE = HW + 4                     # gather source elements (1 zero + 9216 + 3 zeros)
MAGIC = float(1.5 * 2.0 ** 23)

F32 = mybir.dt.float32
BF16 = mybir.dt.float16  # fp16: same speed, 8x mantissa vs bf16; ranges here are tiny
I16 = mybir.dt.int16
AL = mybir.AluOpType
AF = mybir.ActivationFunctionType

# quarter-granularity broadcast units routed to gpsimd partition_broadcast
# instead of broadcast-DMA (load balance Pool vs DMA ring).  72 units total.
N_PB = 18

_CACHE = {}


def _build_program(repeat=1):
    nc = bacc.Bacc("TRN2", target_bir_lowering=False)

    x_d = nc.dram_tensor("x", [C, HW], F32, kind="ExternalInput")
    r_d = nc.dram_tensor("ref", [C, HW], F32, kind="ExternalInput")
    wconv_d = nc.dram_tensor("wconv", [2 * K * C, CO], BF16, kind="ExternalInput")
    wdef_d = nc.dram_tensor("wdef", [K * C, C], BF16, kind="ExternalInput")
    breg_d = nc.dram_tensor("breg", [C, 1], F32, kind="ExternalInput")
    bmod_d = nc.dram_tensor("bmod", [K, 1], F32, kind="ExternalInput")
    maps_d = nc.dram_tensor("maps", [128, NF, 2 * K], F32, kind="ExternalInput")
    id27_d = nc.dram_tensor("id27", [CO, CO], F32, kind="ExternalInput")
    id128_d = nc.dram_tensor("id128", [128, 128], BF16, kind="ExternalInput")
    y_d = nc.dram_tensor("y", [C, HW], F32, kind="ExternalOutput")

    with TileContext(nc) as tc:
        with (
            tc.tile_pool(name="pers", bufs=1) as pers,
            tc.tile_pool(name="dsc", bufs=1, space="DRAM") as dpool,
        ):
            wdef_sb = pers.tile([128, K, C], BF16)
            nc.sync.dma_start(wdef_sb[:], wdef_d[:].rearrange("(a p) o -> p a o", p=128))
            breg_sb = pers.tile([C, 1], F32)
            nc.sync.dma_start(breg_sb[:], breg_d[:])
            id128_sb = pers.tile([128, 128], BF16)
            nc.sync.dma_start(id128_sb[:], id128_d[:])
            rsrc = pers.tile([128, NE, 2], BF16, tag="rsrc")

            cp_dram = dpool.tile([2 * K, HW * 2], BF16, tag="cpd")
            idx_dram = dpool.tile([2 * K, HW], I16, tag="idxd")

            for _rep in range(repeat):
              _wxp_cm = tc.tile_pool(name=f"wx{_rep}", bufs=4)
              wxp = _wxp_cm.__enter__()
              def load_wt(k, ys):
                  wt = wxp.tile([128, HW // 16], I16, tag="widx", name="wt")
                  wsrc = idx_dram[ys * K + k, :].rearrange("(s p) -> p s", p=16)
                  nc.scalar.dma_start(wt[0:16, :], wsrc)
                  p = 16
                  while p < 128:
                      nc.scalar.dma_start(wt[p : 2 * p, :], wt[0:p, :])
                      p *= 2
                  return wt
              wtq = []
              # ---------------- stages A+B+C ----------------
              with (
                tc.tile_pool(name=f"ab{_rep}", bufs=1) as ab,
                tc.tile_pool(name="stg", bufs=4) as stg,
                tc.tile_pool(name="psc", bufs=3, space="PSUM") as pconv,
                tc.tile_pool(name="ppx", bufs=1, space="PSUM") as ppool,
              ):
                wconv_sb = ab.tile([128, 2 * K, CO], BF16)
                nc.sync.dma_start(wconv_sb[:], wconv_d[:].rearrange("(a p) o -> p a o", p=128))
                id27_sb = ab.tile([CO, CO], F32)
                nc.sync.dma_start(id27_sb[:], id27_d[:])
                maps_sb = ab.tile([128, NF, 2 * K], F32)
                nc.sync.dma_start(maps_sb[:], maps_d[:])
                bmk_sb = ab.tile([128, K], F32)
                nc.sync.dma_start(bmk_sb[:], bmod_d[:].rearrange("k a -> a k").to_broadcast((128, K)))

                xc0 = ab.tile([C, PHW], BF16, tag="xc0")
                rpad = ab.tile([C, PHW], BF16, tag="rpad")
                # pad ring zeros (top/bottom rows, left/right cols)
                for t in (xc0, rpad):
                    v = t[:].rearrange("p (h w) -> p h w", h=PH)
                    nc.vector.memset(v[:, 0, :], 0.0)
                    nc.vector.memset(v[:, PH - 1, :], 0.0)
                    nc.vector.memset(v[:, 1 : PH - 1, 0], 0.0)
                    nc.vector.memset(v[:, 1 : PH - 1, PW - 1], 0.0)
                # rsrc ring: element 0 lane 0 and tail elements
                nc.vector.memset(rsrc[:, 0:1, 0], 0.0)
                nc.vector.memset(rsrc[:, HW:NE, :].rearrange("p a b -> p (a b)"), 0.0)

                xpad = xc0[:].rearrange("p (h w) -> p h w", h=PH)
                rpadv = rpad[:].rearrange("p (h w) -> p h w", h=PH)
                NR = 8                      # input rows per staged chunk
                for n in range(H // NR):
                    cb = stg.tile([C, NR, W], F32, tag="cbuf", name="cb")
                    nc.sync.dma_start(cb[:], x_d[:, n * NR * W : (n + 1) * NR * W]
                                      .rearrange("p (h w) -> p h w", h=NR))
                    nc.vector.tensor_copy(xpad[:, 1 + NR * n : 1 + NR * (n + 1), 1 : 1 + W], cb[:])
                    cb2 = stg.tile([C, NR, W], F32, tag="cbuf", name="cb2")
                    q0 = n * NR * W
                    nc.sync.dma_start(cb2[:], r_d[:, q0 : q0 + NR * W]
                                      .rearrange("p (h w) -> p h w", h=NR))
                    nc.vector.tensor_copy(rpadv[:, 1 + NR * n : 1 + NR * (n + 1), 1 : 1 + W], cb2[:])
                    cf = cb2[:].rearrange("p a b -> p (a b)")
                    # pair source: rsrc[i] = (s[i], s[i+1]), s = [0, ref_flat, 0...]
                    nc.vector.tensor_copy(rsrc[:, 1 + q0 : 1 + q0 + NR * W, 0], cf)
                    nc.vector.tensor_copy(rsrc[:, q0 : q0 + NR * W, 1], cf)

                # ---------- conv + transpose into PSUM-resident PPIX ----------
                # inner dim padded to 32 so each transpose's 27-col write stays
                # inside one 128B slot (never straddles a PSUM bank boundary)
                PPIX = ppool.tile([128, NF, 32], F32, tag="PPIX")
                for n in range(NCH):
                    ps = pconv.tile([CO, CHW], F32, tag="convps", name="ps")
                    h0 = n * 4
                    mi = 0
                    for cb_i, xv in enumerate((xpad, rpadv)):
                        for ky in range(KH):
                            for kx in range(KW):
                                rhs = xv[:, h0 + ky : h0 + ky + 4, kx : kx + W]
                                nc.tensor.matmul(
                                    ps[:], wconv_sb[:, cb_i * K + ky * KW + kx, :], rhs,
                                    start=(mi == 0), stop=(mi == 17))
                                mi += 1
                    t27 = stg.tile([CO, CHW], F32, tag="t27", name="t27")
                    nc.scalar.copy(t27[:], ps[:])
                    for s in range(3):
                        nc.tensor.transpose(PPIX[:, n * 3 + s, 0:CO],
                                            t27[:, s * 128 : (s + 1) * 128], id27_sb[:])

                # ---------- stage C: batched pixel pipeline ----------
                def ts1(out, in_, s, op):
                    nc.vector.tensor_scalar(out=out, in0=in_, scalar1=float(s), scalar2=None, op0=op)

                def ts2(out, in_, s1, s2, op0=AL.max, op1=AL.min):
                    nc.vector.tensor_scalar(
                        out=out, in0=in_, scalar1=float(s1), scalar2=float(s2), op0=op0, op1=op1)

                def ct(tag, w=2 * K):
                    return ab.tile([128, NF, w], F32, tag=tag, name=tag)

                # [128, 72, 18] tiles; cols 0..8 = y-taps, 9..17 = x-taps
                p_ = ct("p_")
                nc.vector.tensor_tensor(p_[:], PPIX[:, :, 0 : 2 * K], maps_sb[:], op=AL.add)
                z0 = ct("z0")
                ts2(z0[:], p_[:], MAGIC, MAGIC, AL.add, AL.subtract)
                wf = ct("wf")
                nc.vector.tensor_tensor(wf[:], p_[:], z0[:], op=AL.subtract)
                cl = ct("cl")
                ts2(cl[:], z0[:], 0.0, float(H - 1))
                v0 = ct("v0")
                nc.vector.tensor_tensor(v0[:], z0[:], cl[:], op=AL.is_equal)
                cl1 = ct("cl1")
                ts2(cl1[:], z0[:], -1.0, float(H - 2))
                v1 = ct("v1")
                nc.vector.tensor_tensor(v1[:], z0[:], cl1[:], op=AL.is_equal)
                a0 = ct("a0")
                ts2(a0[:], wf[:], -1.0, 0.5, AL.mult, AL.add)
                nc.vector.tensor_tensor(a0[:], a0[:], v0[:], op=AL.mult)
                a1 = ct("a1")
                nc.vector.scalar_tensor_tensor(
                    out=a1[:], in0=wf[:], scalar=0.5, in1=v1[:], op0=AL.add, op1=AL.mult)

                # modulator 2*sigmoid (2x folded into wdef): per-tap Act with bias
                ms = ct("ms", K)
                for k in range(K):
                    nc.scalar.activation(ms[:, :, k], PPIX[:, :, 2 * K + k], AF.Sigmoid,
                                         bias=bmk_sb[:, k : k + 1])

                # PPIX conv channels are (dy,dx) interleaved per tap: y = even
                # cols, x = odd cols of the 18-wide tiles
                ty0 = ct("ty0", K)
                nc.vector.tensor_tensor(ty0[:], ms[:], a0[:, :, 0 : 2 * K : 2], op=AL.mult)
                ty1 = ct("ty1", K)
                nc.vector.tensor_tensor(ty1[:], ms[:], a1[:, :, 0 : 2 * K : 2], op=AL.mult)
                cpa = [ab.tile([128, NF, K, 2], BF16, tag=f"cpa{ys}", name=f"cpa{ys}")
                       for ys in range(2)]
                for ys, ty in ((0, ty0), (1, ty1)):
                    nc.vector.tensor_tensor(cpa[ys][:, :, :, 0], ty[:], a0[:, :, 1 : 2 * K : 2], op=AL.mult)
                    nc.vector.tensor_tensor(cpa[ys][:, :, :, 1], ty[:], a1[:, :, 1 : 2 * K : 2], op=AL.mult)

                # flat gather indices: if_ys = row_ys*96 + z0x + 1, clamped [0, NE-1]
                iff = [ct(f"if{ys}", K) for ys in range(2)]
                for ys, rows in ((0, cl), (1, cl1)):
                    t = iff[ys]
                    nc.vector.scalar_tensor_tensor(
                        out=t[:], in0=rows[:, :, 0 : 2 * K : 2], scalar=float(W),
                        in1=z0[:, :, 1 : 2 * K : 2], op0=AL.mult, op1=AL.add)
                    # +1 source offset (+97 for row-1 case: rows hold z0y-1's clamp base)
                    off = 1.0 if ys == 0 else float(W + 1)
                    ts2(t[:], t[:], off, 0.0, AL.add, AL.max)
                    ts1(t[:], t[:], float(NE - 1), AL.min)
                iit = []
                for ys in range(2):
                    ii = ab.tile([128, NF, K], I16, tag=f"ii{ys}", name=f"ii{ys}")
                    nc.vector.tensor_copy(ii[:], iff[ys][:])
                    iit.append(ii)
                # idx rows on the Act queue (widx chains follow there), coef
                # rows on SP — two queues drain in parallel, tap-0 first
                for k in range(K):
                    for ys in range(2):
                        dsti = idx_dram[ys * K + k, :].rearrange("(f p) -> p f", p=128)
                        nc.scalar.dma_start(dsti, iit[ys][:, :, k])
                        dst = cp_dram[ys * K + k, :].rearrange("(f p j) -> p f j", p=128, j=2)
                        nc.sync.dma_start(dst, cpa[ys][:, :, k, :])
                    if k == 0:
                        wtq.extend([load_wt(0, 0), load_wt(0, 1)])

              # ---------------- stage D/E: gather, combine, matmul ----------------
              with (
                tc.tile_pool(name=f"gm{_rep}", bufs=2) as gmp,
                tc.tile_pool(name=f"g1{_rep}", bufs=1) as g1p,
                tc.tile_pool(name=f"cr{_rep}", bufs=3) as crp,
                tc.tile_pool(name=f"sc{_rep}", bufs=1) as scp,
                tc.tile_pool(name=f"os{_rep}", bufs=2) as osp,
                tc.tile_pool(name=f"pd{_rep}", bufs=2, space="PSUM") as pdp,
              ):
                partial = scp.tile([128, HW], BF16, tag="partial")
                QW = HW // 4            # quarter pixels (2304)
                PIECES = [(0, 2048), (2048, 2048), (4096, 2048), (6144, 2048), (8192, 1024)]
                for k in range(K):
                    gt = [None, None]
                    # widx chains prefetched one tap ahead; both gathers first
                    # so Pool never waits on pb units
                    for ys in range(2):
                        wt = wtq.pop(0)
                        if k + 1 < K:
                            wtq.append(load_wt(k + 1, ys))
                        pool = gmp if ys == 0 else g1p
                        g = pool.tile([128, HW, 2], BF16, tag="gm" if ys == 0 else "g1",
                                      name=f"g{ys}")
                        gt[ys] = g
                        nc.gpsimd.ap_gather(
                            g[:].rearrange("p a b -> p (a b)").bitcast(F32),
                            rsrc[:].rearrange("p a b -> p (a b)").bitcast(F32), wt[:],
                            channels=128, num_elems=NE, d=1, num_idxs=HW)
                    for ys in range(2):
                        g = gt[ys]
                        for q4 in range(4):
                            crep = crp.tile([128, QW, 2], BF16, tag="crep", name="crep")
                            r_i = ys * K + k
                            rowap = cp_dram[r_i : r_i + 1, q4 * 2 * QW : (q4 + 1) * 2 * QW]
                            # ~3 of 8 quarters replicate on gpsimd, rest on DMA ring
                            if ys == 1 and q4 >= 2:
                                nc.sync.dma_start(
                                    crep[0:1, :].rearrange("p a b -> p (a b)"), rowap)
                                nc.gpsimd.partition_broadcast(
                                    crep[:].rearrange("p a b -> p (a b)"),
                                    crep[0:1, :].rearrange("p a b -> p (a b)"))
                            else:
                                nc.sync.dma_start(
                                    crep[:].rearrange("p a b -> p (a b)"),
                                    rowap.to_broadcast((C, 2 * QW)))
                            if ys == 0 and q4 == 0:
                                hq = QW // 2
                                gl = g[:, 0:hq, :].rearrange("p a b -> p (a b)")
                                nc.vector.tensor_tensor(
                                    gl, gl, crep[:, 0:hq, :].rearrange("p a b -> p (a b)"),
                                    op=AL.mult)
                                gh = g[:, hq:QW, :].rearrange("p a b -> p (a b)")
                                nc.gpsimd.tensor_tensor(
                                    gh, gh, crep[:, hq:QW, :].rearrange("p a b -> p (a b)"),
                                    op=AL.mult)
                            else:
                                gq = g[:, q4 * QW : (q4 + 1) * QW, :].rearrange("p a b -> p (a b)")
                                nc.vector.tensor_tensor(
                                    gq, gq, crep[:].rearrange("p a b -> p (a b)"), op=AL.mult)
                            if ys == 1:
                                # row-sum this quarter immediately (the ys0
                                # counterpart is already multiplied)
                                sl = slice(q4 * QW, (q4 + 1) * QW)
                                m0 = gt[0][:, sl, :].rearrange("p a b -> p (a b)")
                                nc.vector.tensor_tensor(
                                    m0, m0, gt[1][:, sl, :].rearrange("p a b -> p (a b)"),
                                    op=AL.add)
                    # PE contraction; tap accumulation carried in bf16 partial,
                    # re-injected into PSUM via identity matmul
                    for (p0, plen) in PIECES:
                        ps = pdp.tile([128, 2048], F32, tag="dps", name="dps")
                        for q0 in range(0, plen, 512):
                            if k > 0:
                                nc.tensor.matmul(
                                    ps[:, q0 : q0 + 512], id128_sb[:],
                                    partial[:, p0 + q0 : p0 + q0 + 512],
                                    start=True, stop=False)
                            for lane in range(2):
                                nc.tensor.matmul(
                                    ps[:, q0 : q0 + 512], wdef_sb[:, k, :],
                                    gt[0][:, p0 + q0 : p0 + q0 + 512, lane],
                                    start=(lane == 0 and k == 0), stop=(lane == 1))
                        if k < K - 1:
                            nc.scalar.copy(partial[:, p0 : p0 + plen], ps[:, 0:plen])
                        else:
                            for o0 in range(0, plen, 512):
                                ot = osp.tile([128, 512], F32, tag="out", name="ot")
                                nc.scalar.activation(ot[:], ps[:, o0 : o0 + 512], AF.Identity,
                                                     bias=breg_sb[:])
                                nc.sync.dma_start(y_d[:, p0 + o0 : p0 + o0 + 512], ot[:])

              _wxp_cm.__exit__(None, None, None)

    nc.finalize()
    return nc


def _host_maps(b_off):
    q = np.arange(HW)
    p, f = q % 128, q // 128
    hh, ww = (q // W).astype(np.float32), (q % W).astype(np.float32)
    mp = np.zeros((128, NF, 2 * K), np.float32)
    for k in range(K):
        ky, kx = k // KW, k % KW
        mp[p, f, 2 * k] = hh + (ky - 1) + np.float32(b_off[2 * k]) - 0.5
        mp[p, f, 2 * k + 1] = ww + (kx - 1) + np.float32(b_off[2 * k + 1]) - 0.5
    return mp


def kernel(x, ref_feats, w_off, b_off, w_mod, b_mod, w_reg, b_reg):
    if "nc" not in _CACHE:
        _CACHE["nc"] = _build_program()
    nc = _CACHE["nc"]

    w_all = np.concatenate([w_off, w_mod], axis=0).astype(np.float32)
    wc = w_all.reshape(CO, 2, 128, KH, KW).transpose(1, 3, 4, 2, 0)
    wconv = np.ascontiguousarray(wc.reshape(2 * K * C, CO))
    # modulator = 2*sigmoid -> fold the 2x into the deform weights
    wd = (2.0 * np.asarray(w_reg, np.float32)).reshape(C, C, K).transpose(2, 1, 0)
    wdef = np.ascontiguousarray(wd.reshape(K * C, C))

    shared = dict(
        wconv=wconv.astype(np.float16), wdef=wdef.astype(np.float16),
        breg=np.asarray(b_reg, np.float32)[:, None],
        bmod=np.asarray(b_mod, np.float32)[:, None],
        maps=_host_maps(np.asarray(b_off, np.float32)),
        id27=np.eye(CO, dtype=np.float32),
        id128=np.eye(128, dtype=np.float16),
    )
    in_maps = []
    for b in range(B):
        m = dict(shared)
        m["x"] = np.ascontiguousarray(np.asarray(x[b], np.float32).reshape(C, HW))
        m["ref"] = np.ascontiguousarray(np.asarray(ref_feats[b], np.float32).reshape(C, HW))
        in_maps.append(m)
    _CACHE["in_maps"] = in_maps

    res = run_bass_kernel_spmd(nc, in_maps, core_ids=list(range(B)))
    out = np.stack([np.asarray(res.results[b]["y"]).reshape(C, H, W) for b in range(B)])
    return out.astype(np.float32)
